# revision 37
# baseline (speedup 1.0000x reference)
"""BertCorrector kernel for 8 TRN2 NeuronCores.

Computes: segment-mean merge of subword encodings (sorted per-row segment
ids) followed by a dense vocab projection:
    merged[b,w,:] = mean_{s: ids[b,s]==w} enc[b,s,:]   (0 if empty)
    logits = merged @ W + b

Strategy (v14):
  * Globally pack the non-empty (sample, word) pairs (~86.5% of B*WMAX)
    into one contiguous axis and split it evenly across the 8 cores at
    word granularity (7 word-tiles of 128 per core instead of 8).
  * Few, large DMAs.  Host repacks enc to [128 tok, KC, H] and W to
    [128 h, NCH, KO, NV] so enc loads in ~6 grouped issues and each
    1024-wide W chunk in two half issues.  The iota row for the onehot
    build is generated on-device (gpsimd) instead of DMAed.  Issue
    order puts only sc + the stage-A-group-0 enc chunks + W0's first
    half (~1.7MB) ahead of stage B's start; W0's second half and later
    W chunks queue behind the rest of enc (the enc-stream end and the
    B-start are the two arms of the critical path and are balanced).
  * A 9-matmul N=512 warmup bridges the head DMA window and trips the
    PE HAM clock-gate (~3.4us sustained busy) so stage A runs at
    2.4GHz, not 1.2.
  * Stage A (segment sum as enc^T @ onehot) is split into word-groups
    of 256 columns (6 PSUM banks; stage B uses the other 2).  Each
    group's first matmul streams the full group width with start=True
    (clears the bank's has_written bits and initializes every element;
    the toolchain requires per-MM regions uniformly pending or
    written).  Group results cast to mergedT right after the last
    token chunk touching the group retires, so stage B word-tile 0
    starts at ~15us; remaining stage-A groups interleave into stage
    B's tensor stream as enc groups arrive.
  * Stage B is hf-outer per vocab chunk (all 7 word tiles against one
    512-wide W half, then the other half) so each W half has a full
    pass of arrival slack.  PSUM->SBUF casts to bf16 on Vector; one
    merged store per vocab chunk ([p, wt, col] 3D access pattern);
    the final chunk stores per word-tile with the last store issued
    from the idle sync queue to shorten the drain tail.  Host
    upconverts and scatters rows back to [B, WMAX, V] f32.
"""

import numpy as np
import ml_dtypes

B, S, H = 32, 512, 768
V = 8192
WMAX = 256
NCORES = 8
P = 128
KO = H // P          # 6 hidden chunks
NV = 1024            # vocab chunk width
NCH = V // NV        # 8 vocab chunks
NWARM = 9
GW = 2 * P           # stage-A word-group width (aligned to B word tiles)


def _plan(segment_ids):
    """Pack non-empty words globally, split across cores, compute windows."""
    ids = np.asarray(segment_ids, np.int64)
    tok_pid = np.empty((B, S), np.int64)    # global packed word id per token
    packed_rows = []                        # global row index b*WMAX+w per packed word
    counts = []
    base = 0
    for b in range(B):
        u, inv_idx, cnt = np.unique(ids[b], return_inverse=True, return_counts=True)
        tok_pid[b] = base + inv_idx
        packed_rows.append(b * WMAX + u)
        counts.append(cnt)
        base += len(u)
    T = base
    packed_rows = np.concatenate(packed_rows)
    counts = np.concatenate(counts).astype(np.float64)
    flat_pid = tok_pid.ravel()              # nondecreasing

    wbound = np.array([round(c * T / NCORES) for c in range(NCORES + 1)])
    tbound = np.searchsorted(flat_pid, wbound)
    assert tbound[0] == 0 and tbound[-1] == B * S

    nwords = wbound[1:] - wbound[:-1]
    ntoks = tbound[1:] - tbound[:-1]
    WP = int(-(-nwords.max() // P) * P)     # padded packed words per core
    KC = int(-(-ntoks.max() // P))          # token chunks per core

    # per-chunk packed-word windows, unioned over cores
    wins = []
    for kc in range(KC):
        lo, hi = WP, 0
        for c in range(NCORES):
            a = tbound[c] + kc * P
            bnd = min(tbound[c] + (kc + 1) * P, tbound[c + 1])
            if a >= bnd:
                continue
            loc = flat_pid[a:bnd] - wbound[c]
            lo = min(lo, int(loc.min()))
            hi = max(hi, int(loc.max()) + 1)
        wins.append((lo, hi) if lo < hi else None)

    return dict(
        flat_pid=flat_pid, wbound=wbound, tbound=tbound,
        packed_rows=packed_rows, counts=counts, T=T,
        WP=WP, KC=KC, wins=wins,
    )


def _group_plan(plan):
    """Stage-A schedule: word groups of GW columns each (wt-tile aligned).

    Returns groups where groups[q] is a dict with the group's col
    range, the token chunks touching it (each with its clipped col
    range; the first streams the full group width with start=True so
    every PSUM element is initialized), and per-kc onehot col ranges
    are accumulated into plan["oneh_rng"].
    """
    WP, KC, wins = plan["WP"], plan["KC"], plan["wins"]
    ngrp = -(-WP // GW)
    oneh_rng = {}
    groups = []
    for q in range(ngrp):
        qlo, qhi = q * GW, min((q + 1) * GW, WP)
        if qlo >= qhi:
            groups.append(None)
            continue
        items = []
        for kc in range(KC):
            if wins[kc] is None:
                continue
            lo, hi = wins[kc]
            if lo < qhi and hi > qlo:
                if not items:
                    # first toucher streams the full group width so every
                    # later MM lands on uniformly-written PSUM (the
                    # toolchain requires per-MM regions to be uniformly
                    # pending-zero or written)
                    cl, ch = qlo, qhi
                else:
                    cl, ch = max(lo, qlo), min(hi, qhi)
                items.append((kc, cl, ch))
                r = oneh_rng.get(kc)
                oneh_rng[kc] = (cl, ch) if r is None else (min(r[0], cl), max(r[1], ch))
        groups.append(dict(qlo=qlo, qhi=qhi, items=items))
    plan["oneh_rng"] = oneh_rng
    return groups


def _build_program(plan):
    import concourse.mybir as mybir
    from concourse import bacc
    from concourse.tile import TileContext

    bf16 = mybir.dt.bfloat16
    f32 = mybir.dt.float32

    WP, KC = plan["WP"], plan["KC"]
    NWT = WP // P
    groups = _group_plan(plan)
    NGRP = len(groups)
    oneh_rng = plan["oneh_rng"]
    # enc DMA groups: fine-grained at the head (stage-A group 0 needs
    # only the first few chunks before W0), coarser after
    g0_kcs = [it[0] for it in groups[0]["items"]] if groups[0] else [0]
    head_k = max(g0_kcs) + 1
    enc_groups = [(0, (head_k + 1) // 2), ((head_k + 1) // 2, head_k)]
    k = head_k
    while k < KC:
        k2 = min(k + 3, KC)
        enc_groups.append((k, k2))
        k = k2

    nc = bacc.Bacc()
    # per-token constants: [:, :KC] packed word ids, [:, KC:] 1/count
    sc_d = nc.dram_tensor("sc", [P, 2 * KC], f32, kind="ExternalInput")
    enc_d = nc.dram_tensor("enc", [P, KC, H], bf16, kind="ExternalInput")
    w_d = nc.dram_tensor("wmat", [P, NCH, KO, NV], bf16, kind="ExternalInput")
    # word w = wt*128 + p lives at out_d[p, wt, :] (host transposes back)
    out_d = nc.dram_tensor("out", [P, NWT, V], bf16, kind="ExternalOutput")

    with TileContext(nc) as tc:
        with (
            tc.tile_pool(name="persist", bufs=1) as persist,
            tc.tile_pool(name="wp", bufs=3) as wpool,
            tc.tile_pool(name="outp", bufs=2) as outp,
            tc.tile_pool(name="psA", bufs=6, space="PSUM") as psA,
            tc.tile_pool(name="psB", bufs=2, space="PSUM") as psB,
        ):
            # ---- head DMAs: sc, enc groups, W0 halves, rest of enc, W1, W2
            sc_sb = persist.tile([P, 2 * KC], f32)
            nc.sync.dma_start(out=sc_sb[:], in_=sc_d[:])
            # iota row generated on-device: saves 0.46MB of head HBM traffic
            iota_sb = persist.tile([P, WP], f32)
            nc.gpsimd.iota(
                iota_sb[:], [[1, WP]], channel_multiplier=0,
                allow_small_or_imprecise_dtypes=True,
            )
            enc_sb = persist.tile([P, KC, H], bf16)

            def load_enc(g):
                k0, k1 = enc_groups[g]
                nc.sync.dma_start(
                    out=enc_sb[:, k0:k1], in_=enc_d[:, k0:k1],
                )

            w_tiles = {}

            def load_w(n):
                # two half loads: each stage-B hf pass only waits for its
                # own half of the W chunk
                if n < NCH:
                    t = wpool.tile([P, KO, NV], bf16, tag="w")
                    nc.sync.dma_start(
                        out=t[:, :, 0:512], in_=w_d[:, n, :, 0:512])
                    nc.sync.dma_start(
                        out=t[:, :, 512:NV], in_=w_d[:, n, :, 512:NV])
                    w_tiles[n] = t

            # W0's first half lands right when stage A group 0 is cast;
            # its second half (and W1, W2) queue behind the rest of enc —
            # the enc stream end is on the critical path, W0b is not
            # (chunk 0 runs hf0 for all word tiles before touching hf1)
            load_enc(0)
            load_enc(1)
            w0 = wpool.tile([P, KO, NV], bf16, tag="w")
            nc.sync.dma_start(out=w0[:, :, 0:512], in_=w_d[:, 0, :, 0:512])
            w_tiles[0] = w0
            for g in range(2, len(enc_groups)):
                load_enc(g)
            nc.sync.dma_start(out=w0[:, :, 512:NV], in_=w_d[:, 0, :, 512:NV])
            load_w(1)
            load_w(2)

            # ---- PE warmup: trip the HAM clock-gate while DMAs stream ----
            warm_sb = persist.tile([P, 512], bf16)
            nc.vector.memset(warm_sb[:], 0.0)
            warm_ps = psB.tile([P, 512], f32, tag="psB")
            for _ in range(NWARM):
                nc.tensor.matmul(
                    warm_ps[:], lhsT=warm_sb[:, :P], rhs=warm_sb[:],
                    start=True, stop=True,
                )

            # ---- onehot tiles (Vector): oneh[tok, col] = (iota==pid)*inv
            oneh = {}

            def build_oneh(kc):
                if kc in oneh or kc not in oneh_rng:
                    return
                lo, hi = oneh_rng[kc]
                t = persist.tile([P, hi - lo], bf16, name=f"oneh{kc}")
                nc.vector.tensor_scalar(
                    out=t[:],
                    in0=iota_sb[:, lo:hi],
                    scalar1=sc_sb[:, kc:kc + 1],
                    scalar2=sc_sb[:, KC + kc:KC + kc + 1],
                    op0=mybir.AluOpType.is_equal,
                    op1=mybir.AluOpType.mult,
                )
                oneh[kc] = (t, lo)

            # ---- stage A group: mergedT[h, grp cols] = enc^T @ onehot ----
            mergedT = persist.tile([P, KO, WP], bf16)

            def stage_a_group(q):
                g = groups[q]
                if g is None:
                    return
                qlo, qhi, items = g["qlo"], g["qhi"], g["items"]
                if not items:
                    for ko in range(KO):
                        nc.gpsimd.memset(mergedT[:, ko, qlo:qhi], 0.0)
                    return
                for (kc, _, _) in items:
                    build_oneh(kc)
                pts = [psA.tile([P, qhi - qlo], f32, tag="psA", name=f"pa{q}_{ko}")
                       for ko in range(KO)]
                last_kc = items[-1][0]
                for (kc, cl, ch) in items:
                    ot, obase = oneh[kc]
                    for ko in range(KO):
                        nc.tensor.matmul(
                            pts[ko][:, cl - qlo:ch - qlo],
                            lhsT=enc_sb[:, kc, ko * P:(ko + 1) * P],
                            rhs=ot[:, cl - obase:ch - obase],
                            start=(kc == items[0][0]),
                            stop=(kc == last_kc),
                        )
                for ko in range(KO):
                    nc.vector.tensor_copy(
                        out=mergedT[:, ko, qlo:qhi], in_=pts[ko][:, :qhi - qlo],
                    )

            # ---- stage B word tile: out[wt words, chunk n, half hf] ----
            def stage_b_half(n, st, wt, hf):
                pt = psB.tile([P, 512], f32, tag="psB")
                w_sb = w_tiles[n]
                for ko in range(KO):
                    nc.tensor.matmul(
                        pt[:],
                        lhsT=mergedT[:, ko, wt * P:(wt + 1) * P],
                        rhs=w_sb[:, ko, hf * 512:(hf + 1) * 512],
                        start=(ko == 0),
                        stop=(ko == KO - 1),
                    )
                nc.vector.tensor_copy(
                    out=st[:, wt, hf * 512:(hf + 1) * 512], in_=pt[:])

            # wt tile ready once the group fully covering its cols is cast
            # (GW is a multiple of P so each wt tile lies in exactly one group)
            wt_after_grp = {}
            for q in range(NGRP):
                if groups[q] is None:
                    continue
                hi_wt = min(groups[q]["qhi"] // P, NWT)
                for wt in range(NWT):
                    if wt < hi_wt and wt not in wt_after_grp:
                        wt_after_grp[wt] = q
            grp_wts = {q: [wt for wt, qq in wt_after_grp.items() if qq == q]
                       for q in range(NGRP)}

            # ---- interleaved emission: stage A groups feed stage B n=0.
            # hf0 only during the interleave (W0's second half arrives
            # after the enc stream); hf1 follows once stage A is done.
            st0 = outp.tile([P, NWT, NV], bf16, tag="out")
            stage_a_group(0)
            for q in range(1, NGRP):
                for wt in grp_wts.get(q - 1, []):
                    stage_b_half(0, st0, wt, 0)
                stage_a_group(q)
            for wt in grp_wts.get(NGRP - 1, []):
                stage_b_half(0, st0, wt, 0)
            for wt in range(NWT):
                stage_b_half(0, st0, wt, 1)
            nc.scalar.dma_start(out=out_d[:, :, 0:NV], in_=st0[:])

            # ---- stage B remaining chunks: hf-outer so the second half
            # of each W chunk has a full hf0 pass (~9us) of arrival slack
            for n in range(1, NCH):
                load_w(n + 2)
                st = outp.tile([P, NWT, NV], bf16, tag="out")
                for hf in range(NV // 512):
                    for wt in range(NWT):
                        stage_b_half(n, st, wt, hf)
                        if n == NCH - 1 and hf == 1:
                            # last chunk: store per wt to keep the drain
                            # tail short; the final wt's store issues
                            # from the idle sync queue in parallel with
                            # the previous one on scalar
                            eng = nc.sync if wt == NWT - 1 else nc.scalar
                            eng.dma_start(
                                out=out_d[:, wt, n * NV:(n + 1) * NV],
                                in_=st[:, wt],
                            )
                if n < NCH - 1:
                    nc.scalar.dma_start(
                        out=out_d[:, :, n * NV:(n + 1) * NV], in_=st[:])

    nc.finalize()
    return nc


def _prep_inputs(bert_encodings, W, plan):
    flat_pid, wbound, tbound = plan["flat_pid"], plan["wbound"], plan["tbound"]
    counts, WP, KC = plan["counts"], plan["WP"], plan["KC"]

    enc_bf = np.asarray(bert_encodings, dtype=np.float32).reshape(B * S, H)
    enc_bf = enc_bf.astype(ml_dtypes.bfloat16)
    # W: [H, V] -> [P, NCH, KO, NV] (chunk-contiguous per partition)
    w_bf = (np.asarray(W, dtype=np.float32).astype(ml_dtypes.bfloat16)
            .reshape(KO, P, NCH, NV).transpose(1, 2, 0, 3))
    w_bf = np.ascontiguousarray(w_bf)

    inv = (1.0 / counts).astype(np.float32)

    in_maps = []
    for c in range(NCORES):
        t0, t1 = int(tbound[c]), int(tbound[c + 1])
        ntok = t1 - t0
        enc_c = np.zeros((KC * P, H), dtype=ml_dtypes.bfloat16)
        enc_c[:ntok] = enc_bf[t0:t1]
        # [KC, P, H] -> [P, KC, H] (token-within-chunk partition-major)
        enc_c = np.ascontiguousarray(enc_c.reshape(KC, P, H).transpose(1, 0, 2))

        ids_inv = np.zeros((KC * P, 2), dtype=np.float32)
        ids_inv[:, 0] = -1.0
        ids_inv[:ntok, 0] = (flat_pid[t0:t1] - wbound[c]).astype(np.float32)
        ids_inv[:ntok, 1] = inv[flat_pid[t0:t1]]
        ids_inv = ids_inv.reshape(KC, P, 2)

        sc = np.empty((P, 2 * KC), dtype=np.float32)
        sc[:, :KC] = ids_inv[:, :, 0].T
        sc[:, KC:] = ids_inv[:, :, 1].T

        in_maps.append({"sc": sc, "enc": enc_c, "wmat": w_bf})
    return in_maps


def kernel(bert_encodings, segment_ids, W, b, num_words, _trace=False):
    from concourse.bass_utils import run_bass_kernel_spmd

    assert int(num_words) == WMAX
    plan = _plan(segment_ids)
    in_maps = _prep_inputs(bert_encodings, W, plan)
    nc = _build_program(plan)

    core_ids = list(range(NCORES))
    res = run_bass_kernel_spmd(nc, in_maps, core_ids, trace=_trace)

    out = np.zeros((B * WMAX, V), dtype=np.float32)
    wbound, packed_rows = plan["wbound"], plan["packed_rows"]
    for c in core_ids:
        nw = int(wbound[c + 1] - wbound[c])
        arr = np.asarray(res.results[c]["out"])        # [P, NWT, V]
        rows = arr.transpose(1, 0, 2).reshape(-1, V)[:nw].astype(np.float32)
        out[packed_rows[wbound[c]:wbound[c + 1]]] = rows
    out = out.reshape(B, WMAX, V)

    bias = np.asarray(b, dtype=np.float32)
    if np.any(bias):
        out = out + bias

    if _trace:
        kernel._last_exec_time_ns = res.exec_time_ns
        kernel._last_result = res
    return out


# revision 39
# speedup vs baseline: 1.0058x; 1.0058x over previous
"""BertCorrector kernel for 8 TRN2 NeuronCores.

Computes: segment-mean merge of subword encodings (sorted per-row segment
ids) followed by a dense vocab projection:
    merged[b,w,:] = mean_{s: ids[b,s]==w} enc[b,s,:]   (0 if empty)
    logits = merged @ W + b

Strategy (v14):
  * Globally pack the non-empty (sample, word) pairs (~86.5% of B*WMAX)
    into one contiguous axis and split it evenly across the 8 cores at
    word granularity (7 word-tiles of 128 per core instead of 8).
  * Few, large DMAs.  Host repacks enc to [128 tok, KC, H] and W to
    [128 h, NCH, KO, NV] so enc loads in ~6 grouped issues and each
    1024-wide W chunk in two half issues.  The iota row for the onehot
    build is generated on-device (gpsimd) instead of DMAed.  Issue
    order puts only sc + the stage-A-group-0 enc chunks + W0's first
    half (~1.7MB) ahead of stage B's start; W0's second half and later
    W chunks queue behind the rest of enc (the enc-stream end and the
    B-start are the two arms of the critical path and are balanced).
  * A 9-matmul N=512 warmup bridges the head DMA window and trips the
    PE HAM clock-gate (~3.4us sustained busy) so stage A runs at
    2.4GHz, not 1.2.
  * Stage A (segment sum as enc^T @ onehot) is split into word-groups
    of 256 columns (6 PSUM banks; stage B uses the other 2).  Each
    group's first matmul streams the full group width with start=True
    (clears the bank's has_written bits and initializes every element;
    the toolchain requires per-MM regions uniformly pending or
    written).  Group results cast to mergedT right after the last
    token chunk touching the group retires, so stage B word-tile 0
    starts at ~15us; remaining stage-A groups interleave into stage
    B's tensor stream as enc groups arrive.
  * Stage B is hf-outer per vocab chunk (all 7 word tiles against one
    512-wide W half, then the other half) so each W half has a full
    pass of arrival slack.  PSUM->SBUF casts to bf16 on Vector; one
    merged store per vocab chunk ([p, wt, col] 3D access pattern);
    the final chunk stores per word-tile with the last store issued
    from the idle sync queue to shorten the drain tail.  Host
    upconverts and scatters rows back to [B, WMAX, V] f32.
"""

import numpy as np
import ml_dtypes

B, S, H = 32, 512, 768
V = 8192
WMAX = 256
NCORES = 8
P = 128
KO = H // P          # 6 hidden chunks
NV = 1024            # vocab chunk width
NCH = V // NV        # 8 vocab chunks
NWARM = 9
GW = 2 * P           # stage-A word-group width (aligned to B word tiles)


def _plan(segment_ids):
    """Pack non-empty words globally, split across cores, compute windows."""
    ids = np.asarray(segment_ids, np.int64)
    tok_pid = np.empty((B, S), np.int64)    # global packed word id per token
    packed_rows = []                        # global row index b*WMAX+w per packed word
    counts = []
    base = 0
    for b in range(B):
        u, inv_idx, cnt = np.unique(ids[b], return_inverse=True, return_counts=True)
        tok_pid[b] = base + inv_idx
        packed_rows.append(b * WMAX + u)
        counts.append(cnt)
        base += len(u)
    T = base
    packed_rows = np.concatenate(packed_rows)
    counts = np.concatenate(counts).astype(np.float64)
    flat_pid = tok_pid.ravel()              # nondecreasing

    wbound = np.array([round(c * T / NCORES) for c in range(NCORES + 1)])
    tbound = np.searchsorted(flat_pid, wbound)
    assert tbound[0] == 0 and tbound[-1] == B * S

    nwords = wbound[1:] - wbound[:-1]
    ntoks = tbound[1:] - tbound[:-1]
    WP = int(-(-nwords.max() // P) * P)     # padded packed words per core
    KC = int(-(-ntoks.max() // P))          # token chunks per core

    # per-chunk packed-word windows, unioned over cores
    wins = []
    for kc in range(KC):
        lo, hi = WP, 0
        for c in range(NCORES):
            a = tbound[c] + kc * P
            bnd = min(tbound[c] + (kc + 1) * P, tbound[c + 1])
            if a >= bnd:
                continue
            loc = flat_pid[a:bnd] - wbound[c]
            lo = min(lo, int(loc.min()))
            hi = max(hi, int(loc.max()) + 1)
        wins.append((lo, hi) if lo < hi else None)

    return dict(
        flat_pid=flat_pid, wbound=wbound, tbound=tbound,
        packed_rows=packed_rows, counts=counts, T=T,
        WP=WP, KC=KC, wins=wins,
    )


def _group_plan(plan):
    """Stage-A schedule: word groups of GW columns each (wt-tile aligned).

    Returns groups where groups[q] is a dict with the group's col
    range, the token chunks touching it (each with its clipped col
    range; the first streams the full group width with start=True so
    every PSUM element is initialized), and per-kc onehot col ranges
    are accumulated into plan["oneh_rng"].
    """
    WP, KC, wins = plan["WP"], plan["KC"], plan["wins"]
    ngrp = -(-WP // GW)
    oneh_rng = {}
    groups = []
    for q in range(ngrp):
        qlo, qhi = q * GW, min((q + 1) * GW, WP)
        if qlo >= qhi:
            groups.append(None)
            continue
        items = []
        for kc in range(KC):
            if wins[kc] is None:
                continue
            lo, hi = wins[kc]
            if lo < qhi and hi > qlo:
                if not items:
                    # first toucher streams the full group width so every
                    # later MM lands on uniformly-written PSUM (the
                    # toolchain requires per-MM regions to be uniformly
                    # pending-zero or written)
                    cl, ch = qlo, qhi
                else:
                    cl, ch = max(lo, qlo), min(hi, qhi)
                items.append((kc, cl, ch))
                r = oneh_rng.get(kc)
                oneh_rng[kc] = (cl, ch) if r is None else (min(r[0], cl), max(r[1], ch))
        groups.append(dict(qlo=qlo, qhi=qhi, items=items))
    plan["oneh_rng"] = oneh_rng
    return groups


def _build_program(plan):
    import concourse.mybir as mybir
    from concourse import bacc
    from concourse.tile import TileContext

    bf16 = mybir.dt.bfloat16
    f32 = mybir.dt.float32

    WP, KC = plan["WP"], plan["KC"]
    NWT = WP // P
    groups = _group_plan(plan)
    NGRP = len(groups)
    oneh_rng = plan["oneh_rng"]
    # enc DMA groups: fine-grained at the head (stage-A group 0 needs
    # only the first few chunks before W0), coarser after
    g0_kcs = [it[0] for it in groups[0]["items"]] if groups[0] else [0]
    head_k = max(g0_kcs) + 1
    enc_groups = [(0, (head_k + 1) // 2), ((head_k + 1) // 2, head_k)]
    k = head_k
    while k < KC:
        k2 = min(k + 3, KC)
        enc_groups.append((k, k2))
        k = k2
    enc_groups = [(a, b2) for (a, b2) in enc_groups if b2 > a]

    nc = bacc.Bacc()
    # per-token constants: [:, :KC] packed word ids, [:, KC:] 1/count
    sc_d = nc.dram_tensor("sc", [P, 2 * KC], f32, kind="ExternalInput")
    enc_d = nc.dram_tensor("enc", [P, KC, H], bf16, kind="ExternalInput")
    w_d = nc.dram_tensor("wmat", [P, NCH, KO, NV], bf16, kind="ExternalInput")
    # word w = wt*128 + p lives at out_d[p, wt, :] (host transposes back)
    out_d = nc.dram_tensor("out", [P, NWT, V], bf16, kind="ExternalOutput")

    with TileContext(nc) as tc:
        with (
            tc.tile_pool(name="persist", bufs=1) as persist,
            tc.tile_pool(name="wp", bufs=3) as wpool,
            tc.tile_pool(name="outp", bufs=2) as outp,
            tc.tile_pool(name="psA", bufs=6, space="PSUM") as psA,
            tc.tile_pool(name="psB", bufs=2, space="PSUM") as psB,
        ):
            # ---- head DMAs: sc, enc groups, W0 halves, rest of enc, W1, W2
            sc_sb = persist.tile([P, 2 * KC], f32)
            nc.sync.dma_start(out=sc_sb[:], in_=sc_d[:])
            # iota row generated on-device: saves 0.46MB of head HBM traffic
            iota_sb = persist.tile([P, WP], f32)
            nc.gpsimd.iota(
                iota_sb[:], [[1, WP]], channel_multiplier=0,
                allow_small_or_imprecise_dtypes=True,
            )
            enc_sb = persist.tile([P, KC, H], bf16)

            def load_enc(g):
                k0, k1 = enc_groups[g]
                nc.sync.dma_start(
                    out=enc_sb[:, k0:k1], in_=enc_d[:, k0:k1],
                )

            w_tiles = {}

            def load_w(n):
                # two half loads: each stage-B hf pass only waits for its
                # own half of the W chunk
                if n < NCH:
                    t = wpool.tile([P, KO, NV], bf16, tag="w")
                    nc.sync.dma_start(
                        out=t[:, :, 0:512], in_=w_d[:, n, :, 0:512])
                    nc.sync.dma_start(
                        out=t[:, :, 512:NV], in_=w_d[:, n, :, 512:NV])
                    w_tiles[n] = t

            # W0's first half lands right when stage A group 0 is cast;
            # its second half (and W1, W2) queue behind the rest of enc —
            # the enc stream end is on the critical path, W0b is not
            # (chunk 0 runs hf0 for all word tiles before touching hf1)
            load_enc(0)
            if len(enc_groups) > 1:
                load_enc(1)
            w0 = wpool.tile([P, KO, NV], bf16, tag="w")
            nc.sync.dma_start(out=w0[:, :, 0:512], in_=w_d[:, 0, :, 0:512])
            w_tiles[0] = w0
            for g in range(2, len(enc_groups)):
                load_enc(g)
            nc.sync.dma_start(out=w0[:, :, 512:NV], in_=w_d[:, 0, :, 512:NV])
            load_w(1)
            load_w(2)

            # ---- PE warmup: trip the HAM clock-gate while DMAs stream ----
            warm_sb = persist.tile([P, 512], bf16)
            nc.vector.memset(warm_sb[:], 0.0)
            warm_ps = psB.tile([P, 512], f32, tag="psB")
            for _ in range(NWARM):
                nc.tensor.matmul(
                    warm_ps[:], lhsT=warm_sb[:, :P], rhs=warm_sb[:],
                    start=True, stop=True,
                )

            # ---- onehot tiles (Vector): oneh[tok, col] = (iota==pid)*inv
            oneh = {}

            def build_oneh(kc):
                if kc in oneh or kc not in oneh_rng:
                    return
                lo, hi = oneh_rng[kc]
                t = persist.tile([P, hi - lo], bf16, name=f"oneh{kc}")
                nc.vector.tensor_scalar(
                    out=t[:],
                    in0=iota_sb[:, lo:hi],
                    scalar1=sc_sb[:, kc:kc + 1],
                    scalar2=sc_sb[:, KC + kc:KC + kc + 1],
                    op0=mybir.AluOpType.is_equal,
                    op1=mybir.AluOpType.mult,
                )
                oneh[kc] = (t, lo)

            # ---- stage A group: mergedT[h, grp cols] = enc^T @ onehot ----
            mergedT = persist.tile([P, KO, WP], bf16)

            def stage_a_group(q):
                g = groups[q]
                if g is None:
                    return
                qlo, qhi, items = g["qlo"], g["qhi"], g["items"]
                if not items:
                    for ko in range(KO):
                        nc.gpsimd.memset(mergedT[:, ko, qlo:qhi], 0.0)
                    return
                for (kc, _, _) in items:
                    build_oneh(kc)
                pts = [psA.tile([P, qhi - qlo], f32, tag="psA", name=f"pa{q}_{ko}")
                       for ko in range(KO)]
                last_kc = items[-1][0]
                for (kc, cl, ch) in items:
                    ot, obase = oneh[kc]
                    for ko in range(KO):
                        nc.tensor.matmul(
                            pts[ko][:, cl - qlo:ch - qlo],
                            lhsT=enc_sb[:, kc, ko * P:(ko + 1) * P],
                            rhs=ot[:, cl - obase:ch - obase],
                            start=(kc == items[0][0]),
                            stop=(kc == last_kc),
                        )
                for ko in range(KO):
                    nc.vector.tensor_copy(
                        out=mergedT[:, ko, qlo:qhi], in_=pts[ko][:, :qhi - qlo],
                    )

            # ---- stage B word tile: out[wt words, chunk n, half hf] ----
            def stage_b_half(n, st, wt, hf):
                pt = psB.tile([P, 512], f32, tag="psB")
                w_sb = w_tiles[n]
                for ko in range(KO):
                    nc.tensor.matmul(
                        pt[:],
                        lhsT=mergedT[:, ko, wt * P:(wt + 1) * P],
                        rhs=w_sb[:, ko, hf * 512:(hf + 1) * 512],
                        start=(ko == 0),
                        stop=(ko == KO - 1),
                    )
                nc.vector.tensor_copy(
                    out=st[:, wt, hf * 512:(hf + 1) * 512], in_=pt[:])

            # wt tile ready once the group fully covering its cols is cast
            # (GW is a multiple of P so each wt tile lies in exactly one group)
            wt_after_grp = {}
            for q in range(NGRP):
                if groups[q] is None:
                    continue
                hi_wt = min(groups[q]["qhi"] // P, NWT)
                for wt in range(NWT):
                    if wt < hi_wt and wt not in wt_after_grp:
                        wt_after_grp[wt] = q
            grp_wts = {q: [wt for wt, qq in wt_after_grp.items() if qq == q]
                       for q in range(NGRP)}

            # ---- interleaved emission: stage A groups feed stage B n=0.
            # hf0 only during the interleave (W0's second half arrives
            # after the enc stream); hf1 follows once stage A is done.
            st0 = outp.tile([P, NWT, NV], bf16, tag="out")
            stage_a_group(0)
            for q in range(1, NGRP):
                for wt in grp_wts.get(q - 1, []):
                    stage_b_half(0, st0, wt, 0)
                stage_a_group(q)
            for wt in grp_wts.get(NGRP - 1, []):
                stage_b_half(0, st0, wt, 0)
            for wt in range(NWT):
                stage_b_half(0, st0, wt, 1)
            nc.scalar.dma_start(out=out_d[:, :, 0:NV], in_=st0[:])

            # ---- stage B remaining chunks: hf-outer so the second half
            # of each W chunk has a full hf0 pass (~9us) of arrival slack
            for n in range(1, NCH):
                load_w(n + 2)
                st = outp.tile([P, NWT, NV], bf16, tag="out")
                for hf in range(NV // 512):
                    for wt in range(NWT):
                        stage_b_half(n, st, wt, hf)
                        if n == NCH - 1 and hf == 1:
                            # last chunk: store per wt to keep the drain
                            # tail short; the final wt's store issues
                            # from the idle sync queue in parallel with
                            # the previous one on scalar
                            eng = nc.sync if wt == NWT - 1 else nc.scalar
                            eng.dma_start(
                                out=out_d[:, wt, n * NV:(n + 1) * NV],
                                in_=st[:, wt],
                            )
                if n < NCH - 1:
                    nc.scalar.dma_start(
                        out=out_d[:, :, n * NV:(n + 1) * NV], in_=st[:])

    nc.finalize()
    return nc


def _prep_inputs(bert_encodings, W, plan):
    flat_pid, wbound, tbound = plan["flat_pid"], plan["wbound"], plan["tbound"]
    counts, WP, KC = plan["counts"], plan["WP"], plan["KC"]

    enc_bf = np.asarray(bert_encodings, dtype=np.float32).reshape(B * S, H)
    enc_bf = enc_bf.astype(ml_dtypes.bfloat16)
    # W: [H, V] -> [P, NCH, KO, NV] (chunk-contiguous per partition)
    w_bf = (np.asarray(W, dtype=np.float32).astype(ml_dtypes.bfloat16)
            .reshape(KO, P, NCH, NV).transpose(1, 2, 0, 3))
    w_bf = np.ascontiguousarray(w_bf)

    inv = (1.0 / counts).astype(np.float32)

    in_maps = []
    for c in range(NCORES):
        t0, t1 = int(tbound[c]), int(tbound[c + 1])
        ntok = t1 - t0
        enc_c = np.zeros((KC * P, H), dtype=ml_dtypes.bfloat16)
        enc_c[:ntok] = enc_bf[t0:t1]
        # [KC, P, H] -> [P, KC, H] (token-within-chunk partition-major)
        enc_c = np.ascontiguousarray(enc_c.reshape(KC, P, H).transpose(1, 0, 2))

        ids_inv = np.zeros((KC * P, 2), dtype=np.float32)
        ids_inv[:, 0] = -1.0
        ids_inv[:ntok, 0] = (flat_pid[t0:t1] - wbound[c]).astype(np.float32)
        ids_inv[:ntok, 1] = inv[flat_pid[t0:t1]]
        ids_inv = ids_inv.reshape(KC, P, 2)

        sc = np.empty((P, 2 * KC), dtype=np.float32)
        sc[:, :KC] = ids_inv[:, :, 0].T
        sc[:, KC:] = ids_inv[:, :, 1].T

        in_maps.append({"sc": sc, "enc": enc_c, "wmat": w_bf})
    return in_maps


def kernel(bert_encodings, segment_ids, W, b, num_words, _trace=False):
    from concourse.bass_utils import run_bass_kernel_spmd

    assert int(num_words) == WMAX
    plan = _plan(segment_ids)
    in_maps = _prep_inputs(bert_encodings, W, plan)
    nc = _build_program(plan)

    core_ids = list(range(NCORES))
    res = run_bass_kernel_spmd(nc, in_maps, core_ids, trace=_trace)

    out = np.zeros((B * WMAX, V), dtype=np.float32)
    wbound, packed_rows = plan["wbound"], plan["packed_rows"]
    for c in core_ids:
        nw = int(wbound[c + 1] - wbound[c])
        arr = np.asarray(res.results[c]["out"])        # [P, NWT, V]
        rows = arr.transpose(1, 0, 2).reshape(-1, V)[:nw].astype(np.float32)
        out[packed_rows[wbound[c]:wbound[c + 1]]] = rows
    out = out.reshape(B, WMAX, V)

    bias = np.asarray(b, dtype=np.float32)
    if np.any(bias):
        out = out + bias

    if _trace:
        kernel._last_exec_time_ns = res.exec_time_ns
        kernel._last_result = res
    return out
